# revision 17
# baseline (speedup 1.0000x reference)
"""Trainium2 Bass kernel for nn_Decoder (mask-multiply + Linear(512->16) + overlap-add).

Full-input contract: kernel(mixture_w, est_mask, W) -> [4, 128008] float32.

Sharding: 8 cores = 4 batches x 2 K-halves (8000 frames each).

v4: bf16 inputs + packed outputs.
  * The host pre-casts mixture_w/est_mask to bf16 and pre-arranges them into
    the exact SBUF tile layout [4 steps, 128 partitions, 2 tensors, 4 ni,
    2000 frames], so each input DMA is a single 4MB transfer with one
    contiguous 32KB run per partition (~347 GB/s/core measured; the old
    strided f32 layout got ~278 GB/s on twice the bytes).
  * The output stays in k-major [8, 500] form and is DMA'd with 2KB
    contiguous runs per partition into a packed DRAM layout; the host does
    the final (free) transpose to time-major. Writing time-major from the
    device needs 128-byte descriptors, and those tiny out-packets starve the
    input stream: the 16 SDMA engines round-robin between the SP and ACT
    rings at packet granularity (measured +11us/pass of lost input BW).
    This also deletes the PE transposes and ACT ct copies entirely.
  * bf16 rounding costs ~4e-3 relative error, far under the 2e-2 gate.

Per-core raw-bass pipeline (chunk = 500 frames, 16 chunks, 4 DMA steps):
  SP  : one 4MB DMA per step loads [mw; em] into xb[d%4] (4 buffers = 2
        full steps of prefetch slack, so the input ring never starves)
  DVE : per step, one 2x-packed bf16 mult est = mw*em -> eb[d%2];
        per chunk, the overlap-add res[:,k] = A[:,k] + B[:,k-1] (f32 out)
  PE  : per chunk, 4 ni-matmuls (stationary wt [128,40] bf16 with W_A in
        cols 0:8 and W_B in cols 32:40 -- engine APs must start at partition
        0 or 32, and stationary width doesn't change the 500-cycle moving
        cost) -> ps [40,500] f32
  ACT : evacuates the B half ps[32:40] -> sbB (bf16) and issues the 16KB
        k-major output DMA per chunk on its own HWDGE ring
Host unpacks k-major to time-major and adds the 8-sample seam between the
two K-halves of each batch.

Every instruction carries at most one semaphore wait (ISA limit); extra
dependencies are expressed as standalone wait_ge instructions.
"""

import numpy as np
import ml_dtypes

import concourse.bass as bass
import concourse.mybir as mybir
from concourse.bass_utils import run_bass_kernel_spmd

F32 = mybir.dt.float32
BF16 = mybir.dt.bfloat16
BF = ml_dtypes.bfloat16

B, N, K, L = 4, 512, 16000, 16
STEP = L // 2              # 8
KLOC = K // 2              # 8000 frames per core
TLOC = STEP * (KLOC - 1) + L   # 64008 local output samples
CHUNK = 500                # frames per compute chunk
NCHUNK = KLOC // CHUNK     # 16 chunks per pass
KDMA = 4000                # frames per input DMA step
CPD = KDMA // CHUNK        # 8 chunks per DMA step
NDMA = KLOC // KDMA        # 2 DMA steps per pass
XBYTES = 3 * 4 * KDMA      # 48000 input bytes per partition per step


class _Waiter:
    """Absolute-target waits while unrolled; register-advanced inside Fori."""

    def __init__(self, eng):
        self.eng = eng
        self.last = {}
        self.regs = None

    def wait(self, sem, target):
        if self.regs is None:
            self.eng.wait_ge(sem, target)
            self.last[sem.name] = (sem, target)
        else:
            _, prev = self.last[sem.name]
            delta = target - prev
            assert delta >= 0, (sem.name, prev, target)
            self.last[sem.name] = (sem, target)
            reg = self.regs[sem.name]
            if delta:
                self.eng.reg_add(reg, reg, delta)
            self.eng.wait_ge(sem, reg)

    def enter_loop(self):
        self.regs = {}
        for name, (sem, target) in self.last.items():
            reg = self.eng.alloc_register(f"{name}_tgt")
            self.eng.reg_mov(reg, target)
            self.regs[name] = reg


def _build(loops: int | None) -> bass.Bass:
    """loops=None -> graded single-pass kernel (absolute waits only).
    loops>=3 -> bench variant with per-engine Fori steady-state loops."""
    bench = loops is not None
    niter = loops if bench else 1
    G = NCHUNK * niter          # total chunks
    D = NDMA * niter            # total DMA steps
    nc = bass.Bass()
    x = nc.dram_tensor("x", [NDMA, 128, XBYTES], mybir.dt.uint8, kind="ExternalInput")
    wt = nc.dram_tensor("wt", [128, 4, 40], BF16, kind="ExternalInput")
    # packed output: [0:64000] is [chunk s][j, k] (k-major frames, 2KB runs
    # per partition); [64000:64008] is the trailing B half-frame
    out = nc.dram_tensor("out", [TLOC], BF16, kind="ExternalOutput")

    from contextlib import ExitStack

    with ExitStack() as stk:
        e = stk.enter_context
        xb = [e(nc.sbuf_tensor(f"xb{i}", [128, XBYTES], mybir.dt.uint8)) for i in range(2)]
        eb = [e(nc.sbuf_tensor(f"eb{i}", [128, 4, KDMA], BF16)) for i in range(2)]
        wt_sb = e(nc.sbuf_tensor("wt_sb", [128, 4, 40], BF16))
        sbB = e(nc.sbuf_tensor("sbB", [8, NCHUNK * CHUNK], BF16))
        res = [
            e(nc.sbuf_tensor(f"res{i}", [104, NCHUNK // 4, CHUNK], BF16))
            for i in range(2)
        ]
        tail_sb = e(nc.sbuf_tensor("tail_sb", [8, 1], BF16))
        ps = [e(nc.psum_tensor(f"ps{i}", [40, CHUNK], F32)) for i in range(4)]
        wsem = e(nc.semaphore("wsem"))
        dsem = [e(nc.semaphore(f"dsem{i}")) for i in range(2)]
        msem = e(nc.semaphore("msem"))    # DVE mults, +1 per step
        asem = e(nc.semaphore("asem"))    # DVE overlap-adds, +1 per chunk
        psem = e(nc.semaphore("psem"))    # PE matmul groups, +1 per chunk
        esem = e(nc.semaphore("esem"))    # ACT B-half evacs, +1 per chunk
        osem = e(nc.semaphore("osem"))    # ACT out DMAs, +16 per pass
        block = e(nc.Block())

        ET = mybir.EngineType

        def loop_or_unroll(W, engine_type, fn, per_iter):
            """Peel 2 passes then HW-loop (bench), or single pass (graded)."""
            if not bench:
                for i in range(per_iter):
                    fn(i)
                return
            for i in range(2 * per_iter):
                fn(i)
            W.enter_loop()
            with nc.Fori(2, loops, engines=[engine_type]):
                for i in range(per_iter):
                    fn(2 * per_iter + i)

        @block.sync
        def _(sync):
            W = _Waiter(sync)
            sync.dma_start(wt_sb[:], wt[:]).then_inc(wsem, 16)

            def dstep(d):
                if d >= 2:
                    W.wait(msem, d - 1)   # xb[d%2] last read by mult(d-2)
                sync.dma_start(xb[d % 2][:], x[d % NDMA]).then_inc(dsem[d % 2], 16)

            loop_or_unroll(W, ET.SP, dstep, NDMA)
            if bench:
                # two extra steps feed the DVE mult prefetch overrun
                for d2 in (D, D + 1):
                    sync.wait_ge(msem, d2 - 1)
                    sync.dma_start(
                        xb[d2 % 2][:], x[d2 % NDMA]
                    ).then_inc(dsem[d2 % 2], 16)

        @block.vector
        def _(vector):
            W = _Waiter(vector)

            def mult(d):
                W.wait(dsem[d % 2], 16 * (d // 2 + 1))
                if d >= 2:
                    W.wait(psem, CPD * d - CPD)  # eb[d%2] read by step-(d-2) MMs
                nc.vector.tensor_mul(
                    out=eb[d % 2][:].rearrange("p ni k -> p (ni k)"),
                    in0=xb[d % 2][:, 0 : 8 * KDMA].bitcast(BF16),
                    in1=xb[d % 2][:, 8 * KDMA : XBYTES],
                ).then_inc(msem, 1)

            def chunk(g):
                r, s = g // NCHUNK, g % NCHUNK
                par = r % 2
                q0, dd = 32 * (s % 4), s // 4
                W.wait(esem, g + 1)
                if s == 0 and r >= 1:
                    # res[par] read by the pass-(r-2) output DMAs (the r==1
                    # wait is trivially satisfied; it registers the sem for
                    # the Fori register machinery)
                    W.wait(osem, 64 * (r - 1))
                if s >= 1:
                    # one add spanning the chunk boundary via the circular sbB
                    nc.vector.tensor_add(
                        out=res[par][q0 : q0 + 8, dd, 0:CHUNK],
                        in0=ps[g % 4][0:8, 0:CHUNK],
                        in1=sbB[:, CHUNK * s - 1 : CHUNK * s + CHUNK - 1],
                    ).then_inc(asem, 1)
                else:
                    nc.vector.tensor_add(
                        out=res[par][q0 : q0 + 8, dd, 1:CHUNK],
                        in0=ps[g % 4][0:8, 1:CHUNK],
                        in1=sbB[:, 0 : CHUNK - 1],
                    )
                    if g == 0:
                        nc.vector.tensor_copy(
                            out=res[par][q0 : q0 + 8, dd, 0:1],
                            in_=ps[g % 4][0:8, 0:1],
                        ).then_inc(asem, 1)
                    else:
                        nc.vector.tensor_add(
                            out=res[par][q0 : q0 + 8, dd, 0:1],
                            in0=ps[g % 4][0:8, 0:1],
                            in1=sbB[:, NCHUNK * CHUNK - 1 : NCHUNK * CHUNK],
                        ).then_inc(asem, 1)
                if g % CPD == CPD - 1:
                    d = g // CPD + 2
                    if bench or d < NDMA:
                        mult(d)

            mult(0)
            mult(1)
            loop_or_unroll(W, ET.DVE, chunk, NCHUNK)

        @block.tensor
        def _(tensor):
            W = _Waiter(tensor)

            def chunk(g):
                pp = g % 2
                cc = g % CPD
                d2 = (g // CPD) % 2
                if g >= 1:
                    W.wait(msem, g // CPD + 1)  # est of step g//CPD ready
                if g >= 4:
                    W.wait(asem, g - 3)  # ps[g%4] rows 0:8 read by add(g-4)
                for ni in range(4):
                    mm = nc.tensor.matmul(
                        ps[g % 4][:],
                        wt_sb[:, ni],
                        eb[d2][:, ni, cc * CHUNK : (cc + 1) * CHUNK],
                        start=(ni == 0),
                        stop=(ni == 3),
                    )
                    if ni == 3:
                        mm.then_inc(psem, 1)

            tensor.wait_ge(wsem, 16)
            tensor.wait_ge(msem, 1)
            loop_or_unroll(W, ET.PE, chunk, NCHUNK)

        @block.scalar
        def _(scalar):
            W = _Waiter(scalar)

            def chunk(g):
                r, s = g // NCHUNK, g % NCHUNK
                W.wait(psem, g + 1)
                if s == NCHUNK - 1 and g >= NCHUNK:
                    # sbB slot 15 last read by add(g-15); slots 0..14 are
                    # covered by the pass-end out-DMA wait (asem >= 16r)
                    W.wait(asem, g - 14)
                nc.scalar.copy(
                    out=sbB[:, CHUNK * s : CHUNK * (s + 1)], in_=ps[g % 4][32:40]
                ).then_inc(esem, 1)
                if s == NCHUNK - 1:
                    # all adds of pass r landed in res[r%2]: four 8x8KB DMAs,
                    # one per partition group -> SDMA engines 0-3 share the
                    # output work instead of 0/2 carrying all of it
                    W.wait(asem, g + 1)
                    for q in range(4):
                        scalar.dma_start(
                            out[16000 * q : 16000 * (q + 1)].rearrange(
                                "(j m) -> j m", j=8
                            ),
                            res[r % 2][32 * q : 32 * q + 8].rearrange(
                                "j dd k -> j (dd k)"
                            ),
                        ).then_inc(osem, 16)

            loop_or_unroll(W, ET.Activation, chunk, NCHUNK)
            # tail: trailing B half-frame sbB[last][:, CHUNK-1] -> out[64000:]
            scalar.wait_ge(esem, G)
            nc.scalar.copy(
                out=tail_sb[:],
                in_=sbB[:, NCHUNK * CHUNK - 1 : NCHUNK * CHUNK],
            ).then_inc(esem, 1)
            scalar.wait_ge(esem, G + 1)
            scalar.dma_start(
                out[STEP * KLOC : TLOC].rearrange("(j k) -> j k", j=8), tail_sb[:]
            ).then_inc(osem, 16)

    return nc


def build_nc(reps: int = 1) -> bass.Bass:
    return _build(None)


def build_bench_nc(loops: int) -> bass.Bass:
    assert loops >= 3
    return _build(loops)


_NC_CACHE = {}


def _get_nc(reps=1):
    if reps not in _NC_CACHE:
        _NC_CACHE[reps] = _build(None)
    return _NC_CACHE[reps]


def make_in_maps(mixture_w, est_mask, W):
    mwb = (np.asarray(mixture_w, dtype=np.float32) * 2.0**-8).astype(BF)
    emu = np.clip(
        np.rint(np.asarray(est_mask, dtype=np.float32) * 256.0), 0, 255
    ).astype(np.uint8)
    wtt = (
        np.asarray(W, dtype=np.float32)
        .T.reshape(4, 128, L)
        .transpose(1, 0, 2)
        .astype(BF)
    )  # [128, 4, L];  wtt[p, ni, l] = W[l, ni*128 + p]
    wt = np.zeros((128, 4, 40), BF)
    wt[:, :, 0:8] = wtt[:, :, 0:8]     # W_A -> psum partitions 0:8
    wt[:, :, 32:40] = wtt[:, :, 8:16]  # W_B -> psum partitions 32:40
    # mw pre-scaled by 2^-8 (exact in bf16); em quantized to uint8 * 2^-8
    # [b, ni, p, h, d, kk] -> [b, h, d, p, ni, kk]
    M = mwb.reshape(B, 4, 128, 2, NDMA, KDMA).transpose(0, 3, 4, 2, 1, 5)
    E = emu.reshape(B, 4, 128, 2, NDMA, KDMA).transpose(0, 3, 4, 2, 1, 5)
    X = np.empty((B, 2, NDMA, 128, XBYTES), np.uint8)
    X[:, :, :, :, : 8 * KDMA] = np.ascontiguousarray(M).view(np.uint8).reshape(
        B, 2, NDMA, 128, 8 * KDMA
    )
    X[:, :, :, :, 8 * KDMA :] = E.reshape(B, 2, NDMA, 128, 4 * KDMA)
    return [{"x": X[c // 2, c % 2], "wt": wt} for c in range(8)]


def assemble(results):
    T = STEP * (K - 1) + L
    out = np.zeros((B, T), dtype=np.float32)
    for c in range(8):
        b, h = c // 2, c % 2
        v = np.asarray(results[c]["out"], dtype=np.float32)
        body = v[: STEP * KLOC].reshape(4, 8, NCHUNK // 4, CHUNK).transpose(2, 0, 3, 1)
        loc = np.concatenate([body.reshape(-1), v[STEP * KLOC :]])
        out[b, h * STEP * KLOC : h * STEP * KLOC + TLOC] += loc
    return out


def run(mixture_w, est_mask, W, trace=False, reps=1, **spmd_kwargs):
    """Shard, run on 8 cores, gather. Returns (out, BassKernelResults)."""
    in_maps = make_in_maps(mixture_w, est_mask, W)
    nc = _get_nc(reps)
    kr = run_bass_kernel_spmd(
        nc, in_maps, core_ids=list(range(8)), trace=trace, **spmd_kwargs
    )
    return assemble(kr.results), kr


def kernel(mixture_w, est_mask, W):
    out, _ = run(mixture_w, est_mask, W)
    return out


# revision 18
# speedup vs baseline: 1.1748x; 1.1748x over previous
"""Trainium2 Bass kernel for nn_Decoder (mask-multiply + Linear(512->16) + overlap-add).

Full-input contract: kernel(mixture_w, est_mask, W) -> [4, 128008] float32.

Sharding: 8 cores = 4 batches x 2 K-halves (8000 frames each).

v4: bf16 inputs + packed outputs.
  * The host pre-casts mixture_w/est_mask to bf16 and pre-arranges them into
    the exact SBUF tile layout [4 steps, 128 partitions, 2 tensors, 4 ni,
    2000 frames], so each input DMA is a single 4MB transfer with one
    contiguous 32KB run per partition (~347 GB/s/core measured; the old
    strided f32 layout got ~278 GB/s on twice the bytes).
  * The output stays in k-major [8, 500] form and is DMA'd with 2KB
    contiguous runs per partition into a packed DRAM layout; the host does
    the final (free) transpose to time-major. Writing time-major from the
    device needs 128-byte descriptors, and those tiny out-packets starve the
    input stream: the 16 SDMA engines round-robin between the SP and ACT
    rings at packet granularity (measured +11us/pass of lost input BW).
    This also deletes the PE transposes and ACT ct copies entirely.
  * bf16 rounding costs ~4e-3 relative error, far under the 2e-2 gate.

Per-core raw-bass pipeline (chunk = 500 frames, 16 chunks, 4 DMA steps):
  SP  : one 4MB DMA per step loads [mw; em] into xb[d%4] (4 buffers = 2
        full steps of prefetch slack, so the input ring never starves)
  DVE : per step, one 2x-packed bf16 mult est = mw*em -> eb[d%2];
        per chunk, the overlap-add res[:,k] = A[:,k] + B[:,k-1] (f32 out)
  PE  : per chunk, 4 ni-matmuls (stationary wt [128,40] bf16 with W_A in
        cols 0:8 and W_B in cols 32:40 -- engine APs must start at partition
        0 or 32, and stationary width doesn't change the 500-cycle moving
        cost) -> ps [40,500] f32
  ACT : evacuates the B half ps[32:40] -> sbB (bf16) and issues the 16KB
        k-major output DMA per chunk on its own HWDGE ring
Host unpacks k-major to time-major and adds the 8-sample seam between the
two K-halves of each batch.

Every instruction carries at most one semaphore wait (ISA limit); extra
dependencies are expressed as standalone wait_ge instructions.
"""

import numpy as np
import ml_dtypes

import concourse.bass as bass
import concourse.mybir as mybir
from concourse.bass_utils import run_bass_kernel_spmd

F32 = mybir.dt.float32
BF16 = mybir.dt.bfloat16
BF = ml_dtypes.bfloat16

B, N, K, L = 4, 512, 16000, 16
STEP = L // 2              # 8
KLOC = K // 2              # 8000 frames per core
TLOC = STEP * (KLOC - 1) + L   # 64008 local output samples
CHUNK = 500                # frames per compute chunk
NCHUNK = KLOC // CHUNK     # 16 chunks per pass
KDMA = 2000                # frames per input DMA step
CPD = KDMA // CHUNK        # 4 chunks per DMA step
NDMA = KLOC // KDMA        # 4 DMA steps per pass


class _Waiter:
    """Absolute-target waits while unrolled; register-advanced inside Fori."""

    def __init__(self, eng):
        self.eng = eng
        self.last = {}
        self.regs = None

    def wait(self, sem, target):
        if self.regs is None:
            self.eng.wait_ge(sem, target)
            self.last[sem.name] = (sem, target)
        else:
            _, prev = self.last[sem.name]
            delta = target - prev
            assert delta >= 0, (sem.name, prev, target)
            self.last[sem.name] = (sem, target)
            reg = self.regs[sem.name]
            if delta:
                self.eng.reg_add(reg, reg, delta)
            self.eng.wait_ge(sem, reg)

    def enter_loop(self):
        self.regs = {}
        for name, (sem, target) in self.last.items():
            reg = self.eng.alloc_register(f"{name}_tgt")
            self.eng.reg_mov(reg, target)
            self.regs[name] = reg


def _build(loops: int | None) -> bass.Bass:
    """loops=None -> graded single-pass kernel (absolute waits only).
    loops>=3 -> bench variant with per-engine Fori steady-state loops."""
    bench = loops is not None
    niter = loops if bench else 1
    G = NCHUNK * niter          # total chunks
    D = NDMA * niter            # total DMA steps
    nc = bass.Bass()
    x = nc.dram_tensor("x", [NDMA, 128, 2, 4, KDMA], BF16, kind="ExternalInput")
    wt = nc.dram_tensor("wt", [128, 4, 40], BF16, kind="ExternalInput")
    # packed output: [0:64000] is [chunk s][j, k] (k-major frames, 2KB runs
    # per partition); [64000:64008] is the trailing B half-frame
    out = nc.dram_tensor("out", [TLOC], BF16, kind="ExternalOutput")

    from contextlib import ExitStack

    with ExitStack() as stk:
        e = stk.enter_context
        xb = [e(nc.sbuf_tensor(f"xb{i}", [128, 2, 4, KDMA], BF16)) for i in range(4)]
        eb = [e(nc.sbuf_tensor(f"eb{i}", [128, 4, KDMA], BF16)) for i in range(2)]
        wt_sb = e(nc.sbuf_tensor("wt_sb", [128, 4, 40], BF16))
        sbB = [e(nc.sbuf_tensor(f"sbB{i}", [8, CHUNK], BF16)) for i in range(2)]
        res = [
            e(nc.sbuf_tensor(f"res{i}", [104, NCHUNK // 4, CHUNK], BF16))
            for i in range(2)
        ]
        tail_sb = e(nc.sbuf_tensor("tail_sb", [8, 1], BF16))
        ps = [e(nc.psum_tensor(f"ps{i}", [40, CHUNK], F32)) for i in range(4)]
        wsem = e(nc.semaphore("wsem"))
        dsem = [e(nc.semaphore(f"dsem{i}")) for i in range(4)]
        msem = e(nc.semaphore("msem"))    # DVE mults, +1 per step
        asem = e(nc.semaphore("asem"))    # DVE overlap-adds, +1 per chunk
        psem = e(nc.semaphore("psem"))    # PE matmul groups, +1 per chunk
        esem = e(nc.semaphore("esem"))    # ACT B-half evacs, +1 per chunk
        osem = e(nc.semaphore("osem"))    # ACT out DMAs, +16 per pass
        block = e(nc.Block())

        ET = mybir.EngineType

        def loop_or_unroll(W, engine_type, fn, per_iter):
            """Peel 2 passes then HW-loop (bench), or single pass (graded)."""
            if not bench:
                for i in range(per_iter):
                    fn(i)
                return
            for i in range(2 * per_iter):
                fn(i)
            W.enter_loop()
            with nc.Fori(2, loops, engines=[engine_type]):
                for i in range(per_iter):
                    fn(2 * per_iter + i)

        @block.sync
        def _(sync):
            W = _Waiter(sync)
            sync.dma_start(wt_sb[:], wt[:]).then_inc(wsem, 16)

            def dstep(d):
                if d >= 4:
                    W.wait(msem, d - 3)   # xb[d%4] last read by mult(d-4)
                sync.dma_start(xb[d % 4][:], x[d % NDMA]).then_inc(dsem[d % 4], 16)

            loop_or_unroll(W, ET.SP, dstep, NDMA)
            if bench:
                # two extra steps feed the DVE mult prefetch overrun
                for d2 in (D, D + 1):
                    sync.wait_ge(msem, d2 - 3)
                    sync.dma_start(
                        xb[d2 % 4][:], x[d2 % NDMA]
                    ).then_inc(dsem[d2 % 4], 16)

        @block.vector
        def _(vector):
            W = _Waiter(vector)

            def mult(d):
                W.wait(dsem[d % 4], 16 * (d // 4 + 1))
                if d >= 2:
                    W.wait(psem, 4 * d - 4)  # eb[d%2] read by MMs of step d-2
                nc.vector.tensor_mul(
                    out=eb[d % 2][:], in0=xb[d % 4][:, 0], in1=xb[d % 4][:, 1]
                ).then_inc(msem, 1)

            def chunk(g):
                pp = g % 2
                r, s = g // NCHUNK, g % NCHUNK
                par = r % 2
                q0, dd = 32 * (s % 4), s // 4
                W.wait(esem, g + 1)
                if s == 0 and r >= 1:
                    # res[par] read by the pass-(r-2) output DMAs (the r==1
                    # wait is trivially satisfied; it registers the sem for
                    # the Fori register machinery)
                    W.wait(osem, 64 * (r - 1))
                nc.vector.tensor_add(
                    out=res[par][q0 : q0 + 8, dd, 1:CHUNK],
                    in0=ps[g % 4][0:8, 1:CHUNK],
                    in1=sbB[pp][:, 0 : CHUNK - 1],
                )
                if g == 0:
                    nc.vector.tensor_copy(
                        out=res[par][q0 : q0 + 8, dd, 0:1], in_=ps[g % 4][0:8, 0:1]
                    ).then_inc(asem, 1)
                else:
                    nc.vector.tensor_add(
                        out=res[par][q0 : q0 + 8, dd, 0:1],
                        in0=ps[g % 4][0:8, 0:1],
                        in1=sbB[1 - pp][:, CHUNK - 1 : CHUNK],
                    ).then_inc(asem, 1)
                if g % CPD == CPD - 1:
                    d = g // CPD + 2
                    if bench or d < NDMA:
                        mult(d)

            mult(0)
            mult(1)
            loop_or_unroll(W, ET.DVE, chunk, NCHUNK)

        @block.tensor
        def _(tensor):
            W = _Waiter(tensor)

            def chunk(g):
                pp = g % 2
                cc = g % CPD
                d2 = (g // CPD) % 2
                if g >= 1:
                    W.wait(msem, g // CPD + 1)  # est of step g//CPD ready
                if g >= 4:
                    W.wait(asem, g - 3)  # ps[g%4] rows 0:8 read by add(g-4)
                for ni in range(4):
                    mm = nc.tensor.matmul(
                        ps[g % 4][:],
                        wt_sb[:, ni],
                        eb[d2][:, ni, cc * CHUNK : (cc + 1) * CHUNK],
                        start=(ni == 0),
                        stop=(ni == 3),
                    )
                    if ni == 3:
                        mm.then_inc(psem, 1)

            tensor.wait_ge(wsem, 16)
            tensor.wait_ge(msem, 1)
            loop_or_unroll(W, ET.PE, chunk, NCHUNK)

        @block.scalar
        def _(scalar):
            W = _Waiter(scalar)

            def chunk(g):
                pp = g % 2
                r, s = g // NCHUNK, g % NCHUNK
                W.wait(psem, g + 1)
                if g >= 1:
                    W.wait(asem, g)  # sbB[pp] read by add(g-1) boundary
                nc.scalar.copy(out=sbB[pp][:], in_=ps[g % 4][32:40]).then_inc(
                    esem, 1
                )
                if s == NCHUNK - 1:
                    # all adds of pass r landed in res[r%2]: four 8x8KB DMAs,
                    # one per partition group -> SDMA engines 0-3 share the
                    # output work instead of 0/2 carrying all of it
                    W.wait(asem, g + 1)
                    for q in range(4):
                        scalar.dma_start(
                            out[16000 * q : 16000 * (q + 1)].rearrange(
                                "(j m) -> j m", j=8
                            ),
                            res[r % 2][32 * q : 32 * q + 8].rearrange(
                                "j dd k -> j (dd k)"
                            ),
                        ).then_inc(osem, 16)

            loop_or_unroll(W, ET.Activation, chunk, NCHUNK)
            # tail: trailing B half-frame sbB[last][:, CHUNK-1] -> out[64000:]
            scalar.wait_ge(esem, G)
            nc.scalar.copy(
                out=tail_sb[:], in_=sbB[(G - 1) % 2][:, CHUNK - 1 : CHUNK]
            ).then_inc(esem, 1)
            scalar.wait_ge(esem, G + 1)
            scalar.dma_start(
                out[STEP * KLOC : TLOC].rearrange("(j k) -> j k", j=8), tail_sb[:]
            ).then_inc(osem, 16)

    return nc


def build_nc(reps: int = 1) -> bass.Bass:
    return _build(None)


def build_bench_nc(loops: int) -> bass.Bass:
    assert loops >= 3
    return _build(loops)


_NC_CACHE = {}


def _get_nc(reps=1):
    if reps not in _NC_CACHE:
        _NC_CACHE[reps] = _build(None)
    return _NC_CACHE[reps]


def make_in_maps(mixture_w, est_mask, W):
    mwb = np.asarray(mixture_w, dtype=np.float32).astype(BF)
    emb = np.asarray(est_mask, dtype=np.float32).astype(BF)
    wtt = (
        np.asarray(W, dtype=np.float32)
        .T.reshape(4, 128, L)
        .transpose(1, 0, 2)
        .astype(BF)
    )  # [128, 4, L];  wtt[p, ni, l] = W[l, ni*128 + p]
    wt = np.zeros((128, 4, 40), BF)
    wt[:, :, 0:8] = wtt[:, :, 0:8]     # W_A -> psum partitions 0:8
    wt[:, :, 32:40] = wtt[:, :, 8:16]  # W_B -> psum partitions 32:40
    # [b, ni, p, h, d, kk] -> [b, h, d, p, t, ni, kk]
    M = mwb.reshape(B, 4, 128, 2, NDMA, KDMA)
    E = emb.reshape(B, 4, 128, 2, NDMA, KDMA)
    X = np.empty((B, 2, NDMA, 128, 2, 4, KDMA), BF)
    X[:, :, :, :, 0] = M.transpose(0, 3, 4, 2, 1, 5)
    X[:, :, :, :, 1] = E.transpose(0, 3, 4, 2, 1, 5)
    return [{"x": X[c // 2, c % 2], "wt": wt} for c in range(8)]


def assemble(results):
    T = STEP * (K - 1) + L
    out = np.zeros((B, T), dtype=np.float32)
    for c in range(8):
        b, h = c // 2, c % 2
        v = np.asarray(results[c]["out"], dtype=np.float32)
        body = v[: STEP * KLOC].reshape(4, 8, NCHUNK // 4, CHUNK).transpose(2, 0, 3, 1)
        loc = np.concatenate([body.reshape(-1), v[STEP * KLOC :]])
        out[b, h * STEP * KLOC : h * STEP * KLOC + TLOC] += loc
    return out


def run(mixture_w, est_mask, W, trace=False, reps=1, **spmd_kwargs):
    """Shard, run on 8 cores, gather. Returns (out, BassKernelResults)."""
    in_maps = make_in_maps(mixture_w, est_mask, W)
    nc = _get_nc(reps)
    kr = run_bass_kernel_spmd(
        nc, in_maps, core_ids=list(range(8)), trace=trace, **spmd_kwargs
    )
    return assemble(kr.results), kr


def kernel(mixture_w, est_mask, W):
    out, _ = run(mixture_w, est_mask, W)
    return out
